# revision 16
# baseline (speedup 1.0000x reference)
"""Chamfer rate-distortion loss on 8 TRN2 NeuronCores.

Layout: 8 cores = 4 batches x 2 chamfer directions. Each core computes, for
its (batch, direction), the per-point nearest-neighbor squared distance of
8192 query points X against 8192 reference points Y.

Device algorithm per core:
  - X and Y are pre-sorted (host) along coordinate AXIS.
  - matmul trick: lhsT rows = [-2*x0,-2*x1,-2*x2, 1], rhs rows = [y0,y1,y2,|y|^2]
    => PSUM[m,p] = |y_p|^2 - 2 x_m.y_p = D[m,p] - |x_m|^2.  DVE reduce_min along
    the free axis gives min_p (D - |x|^2) per query point (|x|^2 added on host).
  - 64 uniform chunks of 128 sorted queries scan a 1024-wide band of sorted Y
    around their own sorted position (chunk c -> band positions [128c-448,
    128c+576), edges padded with far-away sentinel points).
  - 8 overflow iterations scan the FULL Y for up to 128 "hard" points whose
    nearest neighbor may fall outside their band (selected on host with a
    conservative Morton-neighbor upper bound).

Exactness: for a query x, every Y outside its band differs from x along the
sort axis by at least gap(x), so any excluded point has D >= gap^2.  Host
verifies band_min_D <= gap^2 per point (sound, data-independent); the rare
unverified points are recomputed exactly on host (on expected data: none).
"""

import os

import numpy as np

B, M, P = 4, 8192, 8192
AXIS = 2
NCHUNK = 64          # uniform chunks of 128 sorted queries
BAND = 1024          # uniform band width (columns of rt per chunk)
PAD = 448            # edge-duplicate pad; band = [128c, 128c+1024) in rt cols
NOVER = 8            # overflow iterations: 8 x 1024 = full 8192 scan
NOUT = NCHUNK + NOVER
WT_W = M + 128       # sorted queries + 128 overflow slots
RT_W = P + 2 * PAD   # pad + sorted refs + pad
KROWS = 11           # fp16 hi/lo decomposition rows (see _prep_core)
SCALE = 32.0         # coordinate pre-scale; device min is SCALE^2 * real
LMBDA = 5.0

_CACHE = {}
LAST_RESULTS = None


def _build_bass():
    import concourse.tile as tile
    from concourse import bacc, mybir

    nc = bacc.Bacc(None, target_bir_lowering=False, debug=False)
    f32 = mybir.dt.float32
    f16 = mybir.dt.float16

    wr_d = nc.dram_tensor("wr", [KROWS, WT_W + RT_W], f16, kind="ExternalInput")
    out_d = nc.dram_tensor("out", [128, NOUT], f32, kind="ExternalOutput")

    with tile.TileContext(nc) as tc:
        with (
            tc.tile_pool(name="const", bufs=1) as cpool,
            tc.tile_pool(name="outp", bufs=1) as opool,
            tc.tile_pool(name="psum", bufs=4, space="PSUM") as ppool,
        ):
            wr = cpool.tile([KROWS, WT_W + RT_W], f16)
            nc.sync.dma_start(wr[:], wr_d[:])
            outt = opool.tile([128, NOUT], f32)

            for i in range(NOUT):
                if i < NCHUNK:
                    wcol = 128 * i
                    rcol = 128 * i
                else:
                    wcol = M
                    rcol = PAD + BAND * (i - NCHUNK)
                rcol += WT_W
                ps = ppool.tile([128, BAND], f32)
                lhsT = wr[:, wcol:wcol + 128]
                nc.tensor.matmul(ps[:, 0:512], lhsT, wr[:, rcol:rcol + 512],
                                 start=True, stop=True)
                nc.tensor.matmul(ps[:, 512:1024], lhsT,
                                 wr[:, rcol + 512:rcol + 1024],
                                 start=True, stop=True)
                nc.vector.tensor_reduce(outt[:, i:i + 1], ps[:],
                                        axis=mybir.AxisListType.X,
                                        op=mybir.AluOpType.min)

            nc.sync.dma_start(out_d[:], outt[:])
    nc.compile()
    return nc


def _morton_key(pts):
    rng = pts.max(0) - pts.min(0)
    q = ((pts - pts.min(0)) / (rng + 1e-9) * 1023).astype(np.uint64)

    def spread(x):
        x = x & np.uint64(0x3FF)
        x = (x | (x << np.uint64(16))) & np.uint64(0x30000FF)
        x = (x | (x << np.uint64(8))) & np.uint64(0x300F00F)
        x = (x | (x << np.uint64(4))) & np.uint64(0x30C30C3)
        x = (x | (x << np.uint64(2))) & np.uint64(0x9249249)
        return x

    return (spread(q[:, 0]) | (spread(q[:, 1]) << np.uint64(1))
            | (spread(q[:, 2]) << np.uint64(2)))


def _prep_core(X, Y):
    """Host prep for one (batch, direction): returns in_map plus the metadata
    needed to verify and assemble the result."""
    xo = np.argsort(X[:, AXIS], kind="stable")
    yo = np.argsort(Y[:, AXIS], kind="stable")
    Xs = X[xo]
    Ys = Y[yo]
    X2 = (Xs.astype(np.float64) ** 2).sum(1)
    Y2 = (Ys.astype(np.float64) ** 2).sum(1)
    zx = Xs[:, AXIS].astype(np.float64)
    zy = Ys[:, AXIS].astype(np.float64)

    # gap to nearest excluded Y along the sort axis, per query
    i = np.arange(M)
    c = i // 128
    lo_pos = 128 * c - PAD          # first included Y position
    hi_pos = 128 * c + (BAND - PAD)  # first excluded upper position
    gap = np.full(M, np.inf)
    has_lo = lo_pos > 0
    gap[has_lo] = zx[has_lo] - zy[lo_pos[has_lo] - 1]
    has_hi = hi_pos < P
    gap[has_hi] = np.minimum(gap[has_hi], zy[hi_pos[has_hi]] - zx[has_hi])
    gap = np.maximum(gap, 0.0)

    # conservative NN-distance upper bound via Morton-order neighbors
    allpts = np.concatenate([Xs, Ys]).astype(np.float64)
    mk = _morton_key(allpts)
    inv = np.empty(2 * M, dtype=np.int64)
    inv[np.argsort(mk, kind="stable")] = np.arange(2 * M)
    y_rank = inv[M:]
    order_y = np.argsort(y_rank, kind="stable")
    sorted_ranks = y_rank[order_y]
    K = 16
    idx = np.searchsorted(sorted_ranks, inv[:M])
    cand = np.clip(idx[:, None] + np.arange(-K, K)[None, :], 0, M - 1)
    cands = order_y[cand]
    d2 = ((Xs[:, None, :].astype(np.float64) - Ys[cands].astype(np.float64)) ** 2).sum(-1)
    d_cap2 = d2.min(1)

    hard = np.flatnonzero(~(d_cap2 <= (gap * gap) * 0.98))
    if len(hard) > 128:
        score = np.sqrt(d_cap2[hard]) - gap[hard]
        hard = hard[np.argsort(-score)[:128]]
    over_idx = np.full(128, hard[0] if len(hard) else 0, dtype=np.int64)
    over_idx[:len(hard)] = hard

    # fp16 hi/lo decomposition of SCALE*X and SCALE*Y; device computes
    # SCALE^2 * (|y|^2 - 2 x.y) in fp32 PSUM via K=11 contraction rows:
    #   r0-2: -2*a_d * c_d     r3-5: -2*a_d * e_d     r6-8: -2*b_d * c_d
    #   r9:   1 * w_hi         r10:  1 * w_lo
    # where a+b ~ SCALE*x, c+e ~ SCALE*y, w_hi+w_lo ~ |SCALE*y|^2.
    Xss = (SCALE * Xs).astype(np.float64)
    Yss = (SCALE * Ys).astype(np.float64)
    a = Xss.astype(np.float16)
    bb = (Xss - a.astype(np.float64)).astype(np.float16)
    c = Yss.astype(np.float16)
    e = (Yss - c.astype(np.float64)).astype(np.float16)
    w = (Yss ** 2).sum(1)
    wh = w.astype(np.float16)
    wl = (w - wh.astype(np.float64)).astype(np.float16)

    wr = np.empty((KROWS, WT_W + RT_W), dtype=np.float16)
    wt = wr[:, :WT_W]
    rt = wr[:, WT_W:]

    na = (-2.0 * a.astype(np.float64)).astype(np.float16)  # exact: x2 of fp16
    nb = (-2.0 * bb.astype(np.float64)).astype(np.float16)
    wt[0:3, :M] = na.T
    wt[3:6, :M] = na.T
    wt[6:9, :M] = nb.T
    wt[9:11, :M] = 1.0
    wt[0:3, M:] = na[over_idx].T
    wt[3:6, M:] = na[over_idx].T
    wt[6:9, M:] = nb[over_idx].T
    wt[9:11, M:] = 1.0

    ccT = c.T
    eeT = e.T
    # edge-duplicate padding: repeats of the first/last sorted reference
    # point — real candidates, can never lower a min below the true min.
    for cols, sl in ((slice(0, PAD), 0), (slice(PAD + P, RT_W), P - 1)):
        rt[0:3, cols] = ccT[:, sl:sl + 1]
        rt[3:6, cols] = eeT[:, sl:sl + 1]
        rt[6:9, cols] = ccT[:, sl:sl + 1]
        rt[9, cols] = wh[sl]
        rt[10, cols] = wl[sl]
    rt[0:3, PAD:PAD + P] = ccT
    rt[3:6, PAD:PAD + P] = eeT
    rt[6:9, PAD:PAD + P] = ccT
    rt[9, PAD:PAD + P] = wh
    rt[10, PAD:PAD + P] = wl

    return {"wr": wr}, {
        "Xs": Xs.astype(np.float64), "Ys": Ys.astype(np.float64),
        "X2": X2, "Y2": Y2, "gap": gap, "hard": hard, "over_idx": over_idx,
    }


def _post_core(out, meta):
    """Combine device output into sum over queries of min-D (float64)."""
    inv_s2 = 1.0 / (SCALE * SCALE)
    band_min = out[:, :NCHUNK].T.reshape(M).astype(np.float64) * inv_s2
    dmin = band_min + meta["X2"]

    over_min = out[:, NCHUNK:].min(axis=1).astype(np.float64) * inv_s2
    over_d = over_min + meta["X2"][meta["over_idx"]]
    nhard = len(meta["hard"])
    if nhard:
        dmin[meta["hard"]] = over_d[:nhard]

    # soundness check for band-only points (device fp32 margin included)
    g2 = meta["gap"] * meta["gap"]
    ok = dmin <= g2 - 1e-3 - 1e-3 * np.abs(dmin)
    ok[meta["hard"]] = True
    bad = np.flatnonzero(~ok)
    if len(bad):
        Xb = meta["Xs"][bad]
        db = (meta["Y2"][None, :] - 2.0 * (Xb @ meta["Ys"].T)).min(axis=1)
        dmin[bad] = db + meta["X2"][bad]
    return dmin.sum()


def _install_axon_profile_hook():
    """Make trace=True work under axon when the image's antenv lacks
    axon_hooks: inject a shim module wired to the ctypes NTFF driver."""
    import sys
    import types
    try:
        from antenv.axon_hooks import get_axon_ntff_profile_hook  # noqa: F401
        return
    except ImportError:
        pass
    try:
        import antenv
        from trn_agent_boot.trn_boot import _ntff_profile_via_ctypes
        hook = _ntff_profile_via_ctypes("/opt/axon/libaxon_pjrt.so")
    except Exception:
        hook = None
    mod = types.ModuleType("antenv.axon_hooks")
    state = {"h": hook}
    mod.get_axon_ntff_profile_hook = lambda: state["h"]
    mod.set_axon_ntff_profile_hook = lambda h: state.__setitem__("h", h)
    sys.modules["antenv.axon_hooks"] = mod
    try:
        antenv.axon_hooks = mod
    except Exception:
        pass


def kernel(x_hat, points, likelihoods):
    from concourse.bass_utils import run_bass_kernel_spmd
    global LAST_RESULTS

    trace = bool(int(os.environ.get("CHAMFER_TRACE", "0")))
    if trace:
        _install_axon_profile_hook()

    if "nc" not in _CACHE:
        _CACHE["nc"] = _build_bass()
    nc = _CACHE["nc"]

    in_maps, metas = [], []
    for core in range(8):
        b, d = core // 2, core % 2
        X = x_hat[b] if d == 0 else points[b]
        Y = points[b] if d == 0 else x_hat[b]
        m, meta = _prep_core(np.asarray(X), np.asarray(Y))
        in_maps.append(m)
        metas.append(meta)

    res = run_bass_kernel_spmd(
        nc, in_maps, core_ids=list(range(8)), trace=trace,
    )
    LAST_RESULTS = res

    sums = [_post_core(res.results[c]["out"], metas[c]) for c in range(8)]
    cham_x = sum(sums[c] for c in range(8) if c % 2 == 0) / (B * M)
    cham_y = sum(sums[c] for c in range(8) if c % 2 == 1) / (B * P)
    rec = cham_x + cham_y

    lik = np.asarray(likelihoods, dtype=np.float64)
    bpp = np.log2(lik).sum() / (-(B * P))

    loss = bpp + LMBDA * rec
    return np.array([loss, bpp, rec], dtype=np.float32)


# revision 18
# speedup vs baseline: 1.5780x; 1.5780x over previous
"""Chamfer rate-distortion loss on 8 TRN2 NeuronCores.

Layout: 8 cores = 4 batches x 2 chamfer directions. Each core computes, for
its (batch, direction), the per-point nearest-neighbor squared distance of
8192 query points X against 8192 reference points Y.

Device algorithm per core:
  - X and Y are pre-sorted (host) along coordinate AXIS.
  - matmul trick: lhsT rows = [-2*x0,-2*x1,-2*x2, 1], rhs rows = [y0,y1,y2,|y|^2]
    => PSUM[m,p] = |y_p|^2 - 2 x_m.y_p = D[m,p] - |x_m|^2.  DVE reduce_min along
    the free axis gives min_p (D - |x|^2) per query point (|x|^2 added on host).
  - 64 uniform chunks of 128 sorted queries scan a 1024-wide band of sorted Y
    around their own sorted position (chunk c -> band positions [128c-448,
    128c+576), edges padded with far-away sentinel points).
  - 8 overflow iterations scan the FULL Y for up to 128 "hard" points whose
    nearest neighbor may fall outside their band (selected on host with a
    conservative Morton-neighbor upper bound).

Exactness: for a query x, every Y outside its band differs from x along the
sort axis by at least gap(x), so any excluded point has D >= gap^2.  Host
verifies band_min_D <= gap^2 per point (sound, data-independent); the rare
unverified points are recomputed exactly on host (on expected data: none).
"""

import os

import numpy as np

B, M, P = 4, 8192, 8192
AXIS = 2
NCHUNK = 64          # uniform chunks of 128 sorted queries
BAND = 512           # uniform band width (columns of rt per chunk)
PAD = 192            # edge-duplicate pad; band = [128c, 128c+512) in rt cols
GRP = 4              # chunks per PSUM tile / per reduce op
NOVER = 16           # overflow windows of 512: full 8192 scan
NOUT = NCHUNK + NOVER
WT_W = M + 128       # sorted queries + 128 overflow slots
RT_W = P + 2 * PAD   # pad + sorted refs + pad
KROWS = 11           # fp16 hi/lo decomposition rows (see _prep_core)
SCALE = 32.0         # coordinate pre-scale; device min is SCALE^2 * real
LMBDA = 5.0

_CACHE = {}
LAST_RESULTS = None


def _build_bass():
    import concourse.tile as tile
    from concourse import bacc, mybir

    nc = bacc.Bacc(None, target_bir_lowering=False, debug=False)
    f32 = mybir.dt.float32
    f16 = mybir.dt.float16

    wr_d = nc.dram_tensor("wr", [KROWS, WT_W + RT_W], f16, kind="ExternalInput")
    out_d = nc.dram_tensor("out", [128, NOUT], f32, kind="ExternalOutput")

    with tile.TileContext(nc) as tc:
        with (
            tc.tile_pool(name="const", bufs=1) as cpool,
            tc.tile_pool(name="outp", bufs=1) as opool,
            tc.tile_pool(name="psum", bufs=2, space="PSUM") as ppool,
        ):
            wr = cpool.tile([KROWS, WT_W + RT_W], f16)
            # split the input DMA so early chunks' data lands first
            for q in range(4):
                nc.sync.dma_start(wr[:, 2048 * q:2048 * (q + 1)],
                                  wr_d[:, 2048 * q:2048 * (q + 1)])
            nc.sync.dma_start(wr[:, M:WT_W], wr_d[:, M:WT_W])
            for q in range(4):
                a, b = WT_W + 2048 * q, WT_W + 2048 * (q + 1)
                nc.sync.dma_start(wr[:, a:b], wr_d[:, a:b])
            nc.sync.dma_start(wr[:, WT_W + P:], wr_d[:, WT_W + P:])
            outt = opool.tile([128, NOUT], f32)

            for k in range(NOUT // GRP):
                ps = ppool.tile([128, GRP, BAND], f32)
                for g in range(GRP):
                    i = GRP * k + g
                    if i < NCHUNK:
                        wcol = 128 * i
                        rcol = 128 * i
                    else:
                        wcol = M
                        rcol = PAD + BAND * (i - NCHUNK)
                    nc.tensor.matmul(ps[:, g, :], wr[:, wcol:wcol + 128],
                                     wr[:, WT_W + rcol:WT_W + rcol + BAND],
                                     start=True, stop=True)
                nc.vector.tensor_reduce(outt[:, GRP * k:GRP * (k + 1)], ps[:],
                                        axis=mybir.AxisListType.X,
                                        op=mybir.AluOpType.min)

            nc.sync.dma_start(out_d[:], outt[:])
    nc.compile()
    return nc


def _morton_key(pts):
    rng = pts.max(0) - pts.min(0)
    q = ((pts - pts.min(0)) / (rng + 1e-9) * 1023).astype(np.uint64)

    def spread(x):
        x = x & np.uint64(0x3FF)
        x = (x | (x << np.uint64(16))) & np.uint64(0x30000FF)
        x = (x | (x << np.uint64(8))) & np.uint64(0x300F00F)
        x = (x | (x << np.uint64(4))) & np.uint64(0x30C30C3)
        x = (x | (x << np.uint64(2))) & np.uint64(0x9249249)
        return x

    return (spread(q[:, 0]) | (spread(q[:, 1]) << np.uint64(1))
            | (spread(q[:, 2]) << np.uint64(2)))


def _prep_core(X, Y):
    """Host prep for one (batch, direction): returns in_map plus the metadata
    needed to verify and assemble the result."""
    xo = np.argsort(X[:, AXIS], kind="stable")
    yo = np.argsort(Y[:, AXIS], kind="stable")
    Xs = X[xo]
    Ys = Y[yo]
    X2 = (Xs.astype(np.float64) ** 2).sum(1)
    Y2 = (Ys.astype(np.float64) ** 2).sum(1)
    zx = Xs[:, AXIS].astype(np.float64)
    zy = Ys[:, AXIS].astype(np.float64)

    # gap to nearest excluded Y along the sort axis, per query
    i = np.arange(M)
    c = i // 128
    lo_pos = 128 * c - PAD          # first included Y position
    hi_pos = 128 * c + (BAND - PAD)  # first excluded upper position
    gap = np.full(M, np.inf)
    has_lo = lo_pos > 0
    gap[has_lo] = zx[has_lo] - zy[lo_pos[has_lo] - 1]
    has_hi = hi_pos < P
    gap[has_hi] = np.minimum(gap[has_hi], zy[hi_pos[has_hi]] - zx[has_hi])
    gap = np.maximum(gap, 0.0)

    # conservative NN-distance upper bound via Morton-order neighbors
    allpts = np.concatenate([Xs, Ys]).astype(np.float64)
    mk = _morton_key(allpts)
    inv = np.empty(2 * M, dtype=np.int64)
    inv[np.argsort(mk, kind="stable")] = np.arange(2 * M)
    y_rank = inv[M:]
    order_y = np.argsort(y_rank, kind="stable")
    sorted_ranks = y_rank[order_y]
    K = 16
    idx = np.searchsorted(sorted_ranks, inv[:M])
    cand = np.clip(idx[:, None] + np.arange(-K, K)[None, :], 0, M - 1)
    cands = order_y[cand]
    d2 = ((Xs[:, None, :].astype(np.float64) - Ys[cands].astype(np.float64)) ** 2).sum(-1)
    d_cap2 = d2.min(1)

    hard = np.flatnonzero(~(d_cap2 <= (gap * gap) * 0.98))
    if len(hard) > 128:
        score = np.sqrt(d_cap2[hard]) - gap[hard]
        hard = hard[np.argsort(-score)[:128]]
    over_idx = np.full(128, hard[0] if len(hard) else 0, dtype=np.int64)
    over_idx[:len(hard)] = hard

    # fp16 hi/lo decomposition of SCALE*X and SCALE*Y; device computes
    # SCALE^2 * (|y|^2 - 2 x.y) in fp32 PSUM via K=11 contraction rows:
    #   r0-2: -2*a_d * c_d     r3-5: -2*a_d * e_d     r6-8: -2*b_d * c_d
    #   r9:   1 * w_hi         r10:  1 * w_lo
    # where a+b ~ SCALE*x, c+e ~ SCALE*y, w_hi+w_lo ~ |SCALE*y|^2.
    Xss = (SCALE * Xs).astype(np.float64)
    Yss = (SCALE * Ys).astype(np.float64)
    a = Xss.astype(np.float16)
    bb = (Xss - a.astype(np.float64)).astype(np.float16)
    c = Yss.astype(np.float16)
    e = (Yss - c.astype(np.float64)).astype(np.float16)
    w = (Yss ** 2).sum(1)
    wh = w.astype(np.float16)
    wl = (w - wh.astype(np.float64)).astype(np.float16)

    wr = np.empty((KROWS, WT_W + RT_W), dtype=np.float16)
    wt = wr[:, :WT_W]
    rt = wr[:, WT_W:]

    na = (-2.0 * a.astype(np.float64)).astype(np.float16)  # exact: x2 of fp16
    nb = (-2.0 * bb.astype(np.float64)).astype(np.float16)
    wt[0:3, :M] = na.T
    wt[3:6, :M] = na.T
    wt[6:9, :M] = nb.T
    wt[9:11, :M] = 1.0
    wt[0:3, M:] = na[over_idx].T
    wt[3:6, M:] = na[over_idx].T
    wt[6:9, M:] = nb[over_idx].T
    wt[9:11, M:] = 1.0

    ccT = c.T
    eeT = e.T
    # edge-duplicate padding: repeats of the first/last sorted reference
    # point — real candidates, can never lower a min below the true min.
    for cols, sl in ((slice(0, PAD), 0), (slice(PAD + P, RT_W), P - 1)):
        rt[0:3, cols] = ccT[:, sl:sl + 1]
        rt[3:6, cols] = eeT[:, sl:sl + 1]
        rt[6:9, cols] = ccT[:, sl:sl + 1]
        rt[9, cols] = wh[sl]
        rt[10, cols] = wl[sl]
    rt[0:3, PAD:PAD + P] = ccT
    rt[3:6, PAD:PAD + P] = eeT
    rt[6:9, PAD:PAD + P] = ccT
    rt[9, PAD:PAD + P] = wh
    rt[10, PAD:PAD + P] = wl

    return {"wr": wr}, {
        "Xs": Xs.astype(np.float64), "Ys": Ys.astype(np.float64),
        "X2": X2, "Y2": Y2, "gap": gap, "hard": hard, "over_idx": over_idx,
    }


def _post_core(out, meta):
    """Combine device output into sum over queries of min-D (float64)."""
    inv_s2 = 1.0 / (SCALE * SCALE)
    band_min = out[:, :NCHUNK].T.reshape(M).astype(np.float64) * inv_s2
    dmin = band_min + meta["X2"]

    over_min = out[:, NCHUNK:].min(axis=1).astype(np.float64) * inv_s2
    over_d = over_min + meta["X2"][meta["over_idx"]]
    nhard = len(meta["hard"])
    if nhard:
        dmin[meta["hard"]] = over_d[:nhard]

    # soundness check for band-only points (device fp32 margin included)
    g2 = meta["gap"] * meta["gap"]
    ok = dmin <= g2 - 1e-3 - 1e-3 * np.abs(dmin)
    ok[meta["hard"]] = True
    bad = np.flatnonzero(~ok)
    if len(bad):
        Xb = meta["Xs"][bad]
        db = (meta["Y2"][None, :] - 2.0 * (Xb @ meta["Ys"].T)).min(axis=1)
        dmin[bad] = db + meta["X2"][bad]
    return dmin.sum()


def _install_axon_profile_hook():
    """Make trace=True work under axon when the image's antenv lacks
    axon_hooks: inject a shim module wired to the ctypes NTFF driver."""
    import sys
    import types
    try:
        from antenv.axon_hooks import get_axon_ntff_profile_hook  # noqa: F401
        return
    except ImportError:
        pass
    try:
        import antenv
        from trn_agent_boot.trn_boot import _ntff_profile_via_ctypes
        hook = _ntff_profile_via_ctypes("/opt/axon/libaxon_pjrt.so")
    except Exception:
        hook = None
    mod = types.ModuleType("antenv.axon_hooks")
    state = {"h": hook}
    mod.get_axon_ntff_profile_hook = lambda: state["h"]
    mod.set_axon_ntff_profile_hook = lambda h: state.__setitem__("h", h)
    sys.modules["antenv.axon_hooks"] = mod
    try:
        antenv.axon_hooks = mod
    except Exception:
        pass


def kernel(x_hat, points, likelihoods):
    from concourse.bass_utils import run_bass_kernel_spmd
    global LAST_RESULTS

    trace = bool(int(os.environ.get("CHAMFER_TRACE", "0")))
    if trace:
        _install_axon_profile_hook()

    if "nc" not in _CACHE:
        _CACHE["nc"] = _build_bass()
    nc = _CACHE["nc"]

    in_maps, metas = [], []
    for core in range(8):
        b, d = core // 2, core % 2
        X = x_hat[b] if d == 0 else points[b]
        Y = points[b] if d == 0 else x_hat[b]
        m, meta = _prep_core(np.asarray(X), np.asarray(Y))
        in_maps.append(m)
        metas.append(meta)

    res = run_bass_kernel_spmd(
        nc, in_maps, core_ids=list(range(8)), trace=trace,
    )
    LAST_RESULTS = res

    sums = [_post_core(res.results[c]["out"], metas[c]) for c in range(8)]
    cham_x = sum(sums[c] for c in range(8) if c % 2 == 0) / (B * M)
    cham_y = sum(sums[c] for c in range(8) if c % 2 == 1) / (B * P)
    rec = cham_x + cham_y

    lik = np.asarray(likelihoods, dtype=np.float64)
    bpp = np.log2(lik).sum() / (-(B * P))

    loss = bpp + LMBDA * rec
    return np.array([loss, bpp, rec], dtype=np.float32)
